# revision 2
# baseline (speedup 1.0000x reference)
"""Trainium2 Bass kernel for CosineSimCodebook eval forward.

Problem (hardcoded): x [8, 4096, 512] f32, embeddings [1, 8192, 512] f32.
Returns (quantize [8,4096,512] f32, embed_ind [8,4096] i32, dist [1,8,4096,8192] f32).

Sharding: data-parallel over batch dim b (8 cores, one batch slice each);
codebook replicated. No collectives needed (eval mode).

Per-core device kernel:
  dist = x_b @ e.T   on PE in float32r (TF32-like fp32 mode, ~3.2x faster
                     than fp32; measured rel err ~1.4e-4 per dot product)
  top-8 + argmax     via DVE max/max_index (full fp32 PSUM values)
  quantize = e[idx]  via gpsimd indirect-DMA gather

Host side: inputs are pre-transposed (xT [512,4096], eT [512,8192]) so the
contraction dim lands on SBUF partitions with natural-layout DMAs. Because
f32r rounds operands, tokens whose top-2 distance gap is below a threshold
(~30 sigma of the f32r matmul noise) get their argmax recomputed exactly on
host from the device's top-8 candidate codes; everything else is provably
stable under the noise bound.
"""

import numpy as np

B, N, D, C, P = 8, 4096, 512, 8192, 128
NT = N // P   # 32 token tiles per core
KC = D // P   # 4 contraction chunks
FD = 512      # matmul free dim (one PSUM bank of f32)
NG = C // FD  # 16 code groups

MODE = "f32r"      # "f32r" or "fp32"
GAP_THRESH = 4e-3  # host fix-up threshold on top1-top2 gap (f32r noise ~2e-4 rms)

_cache = {}


def _build():
    import concourse.bacc as bacc
    import concourse.tile as tile
    from concourse import bass, mybir

    f32 = mybir.dt.float32
    u32 = mybir.dt.uint32
    mm_dt = f32 if MODE == "fp32" else mybir.dt.float32r

    nc = bacc.Bacc(None, target_bir_lowering=False, debug=False)

    xT = nc.dram_tensor("xT", [D, N], mm_dt, kind="ExternalInput")
    eT = nc.dram_tensor("eT", [D, C], mm_dt, kind="ExternalInput")
    etab = nc.dram_tensor("etab", [C, D], f32, kind="ExternalInput")
    dist = nc.dram_tensor("dist", [N, C], f32, kind="ExternalOutput")
    qout = nc.dram_tensor("qout", [N, D], f32, kind="ExternalOutput")
    iout = nc.dram_tensor("iout", [N, 8], u32, kind="ExternalOutput")
    vout = nc.dram_tensor("vout", [N, 8], f32, kind="ExternalOutput")

    with tile.TileContext(nc) as tc:
        with (
            tc.tile_pool(name="econst", bufs=1) as epool,
            tc.tile_pool(name="xp", bufs=3) as xpool,
            tc.tile_pool(name="dp", bufs=2) as dpool,
            tc.tile_pool(name="qp", bufs=2) as qpool,
            tc.tile_pool(name="mx", bufs=2) as mpool,
            tc.tile_pool(name="ps", bufs=8, space="PSUM") as pspool,
        ):
            eT_sb = epool.tile([P, KC, C], mm_dt)
            for k in range(KC):
                nc.sync.dma_start(eT_sb[:, k, :], eT[k * P:(k + 1) * P, :])

            for m in range(NT):
                ms = slice(m * P, (m + 1) * P)
                xT_sb = xpool.tile([P, KC, P], mm_dt)
                for k in range(KC):
                    nc.sync.dma_start(xT_sb[:, k, :], xT[k * P:(k + 1) * P, ms])

                dist_sb = dpool.tile([P, C], f32)
                for g in range(NG):
                    gs = slice(g * FD, (g + 1) * FD)
                    ps = pspool.tile([P, FD], f32)
                    for k in range(KC):
                        nc.tensor.matmul(
                            ps[:],
                            xT_sb[:, k, :],
                            eT_sb[:, k, gs],
                            start=(k == 0),
                            stop=(k == KC - 1),
                        )
                    nc.scalar.copy(dist_sb[:, gs], ps[:])
                    nc.sync.dma_start(dist[ms, gs], dist_sb[:, gs])

                max8 = mpool.tile([P, 8], f32)
                idx8 = mpool.tile([P, 8], u32)
                nc.vector.max(max8[:], dist_sb[:])
                nc.vector.max_index(idx8[:], max8[:], dist_sb[:])
                nc.sync.dma_start(iout[ms, :], idx8[:])
                nc.sync.dma_start(vout[ms, :], max8[:])

                q_sb = qpool.tile([P, D], f32)
                nc.gpsimd.indirect_dma_start(
                    out=q_sb[:],
                    out_offset=None,
                    in_=etab[:],
                    in_offset=bass.IndirectOffsetOnAxis(ap=idx8[:, :1], axis=0),
                )
                nc.sync.dma_start(qout[ms, :], q_sb[:])

    nc.compile()
    return nc


def _fixup(x_flat, e, embed_ind, quantize, idx8, max8):
    """Recompute argmax exactly (f64) for tokens whose top-2 gap is within
    the f32r noise band. Candidates = device top-8 codes."""
    gap = max8[:, 0] - max8[:, 1]
    flagged = np.nonzero(gap < GAP_THRESH)[0]
    for t in flagged:
        cand = idx8[t]
        cand = cand[cand < C].astype(np.int64)
        exact = e[cand].astype(np.float64) @ x_flat[t].astype(np.float64)
        best = int(cand[int(np.argmax(exact))])
        embed_ind[t] = best
        quantize[t] = e[best]
    return len(flagged)


def run(x, embeddings, trace=False):
    from concourse.bass_utils import run_bass_kernel_spmd

    if "nc" not in _cache:
        _cache["nc"] = _build()
    nc = _cache["nc"]

    x = np.asarray(x, dtype=np.float32)
    e = np.ascontiguousarray(np.asarray(embeddings, dtype=np.float32)[0])  # [C, D]
    eT = np.ascontiguousarray(e.T)  # [D, C]

    in_maps = [
        {"xT": np.ascontiguousarray(x[b].T), "eT": eT, "etab": e}
        for b in range(B)
    ]
    out = run_bass_kernel_spmd(nc, in_maps, list(range(B)), trace=trace)
    res = out.results

    dist = np.stack([res[b]["dist"] for b in range(B)])[None]        # [1,8,N,C]
    embed_ind = np.stack([res[b]["iout"][:, 0] for b in range(B)]).astype(np.int64)
    quantize = np.stack([res[b]["qout"] for b in range(B)])

    if MODE != "fp32":
        nfix = 0
        for b in range(B):
            nfix += _fixup(
                x[b], e, embed_ind[b], quantize[b],
                res[b]["iout"], res[b]["vout"],
            )
        run.last_nfix = nfix

    return (quantize, embed_ind.astype(np.int32), dist), out


def kernel(x, embeddings):
    (quantize, embed_ind, dist), _ = run(x, embeddings, trace=False)
    return quantize, embed_ind, dist
